# revision 1
# baseline (speedup 1.0000x reference)
"""Trainium2 Bass kernel for nn_MultiHeadAttention_46093589021200.

Causal MHA: B=4, S=2048, E=1024, H=16, D=64, with the reference's
"no-transpose-back" reshape (b,h,s,d)->(b,s,e) before the output projection.

Sharding: pure head-parallel, 2 heads per core, zero collectives.
Because of the reshape quirk, output rows s' in [h*128,(h+1)*128) depend only
on head h, so each core produces two independent 128-row output bands per
batch.

Device algorithm (per core, fp16 compute / fp32 PSUM accumulation):
  - qkvT = Wqkv_c^T @ x^T computed directly in head-major [col, s] layout
    (x is passed pre-transposed+pre-cast from the host; contraction over e
    in 8 PSUM-accumulated K=128 chunks).
  - v transposed to [s, d] via the DMA xbar, augmented with a ones column
    per head so the PV matmul also produces softmax denominators (M=65).
  - scoresT[k,q] per 128-k chunk on PE, two heads packed into row groups
    0-1 / 2-3 of the systolic array (K=64 each, concurrent).
  - exp on ACT, one instruction covering both heads per chunk
    (scale=1/sqrt(D) folded in); causality = skipping k>q chunks entirely
    plus a triangular fp16 mask multiply on diagonal chunks (no
    max-subtraction needed: scores/sqrt(D) ~ N(0,1)).
  - PV accumulates att_aug[d+1, q] in PSUM with v_aug stationary; DVE
    reciprocal of the rowsum row + GPSIMD partition_broadcast + one DVE
    multiply produce normalized fp16 attn.
  - o_proj consumes attn through a stride-16 AP view, which implements the
    reference's (b,h,s,d)->(b,s,e) reshape exactly; head 1's attn rows are
    DMA-moved to partitions 64-127 so the two heads' K=64 o_proj matmuls
    row-pack; bias added via a K=1 ones matmul.

NOTE: column-positioned matmuls (tile_position=(0,32j), PSUM output at a
partition offset) mis-execute on this hardware path even though CoreSim
accepts them — this kernel uses row-group packing only.
"""

import sys

if "/opt/trn_rl_repo" not in sys.path:
    sys.path.insert(0, "/opt/trn_rl_repo")

import numpy as np

B, S, E, H = 4, 2048, 1024, 16
D = E // H          # 64
NCORES = 8
HPC = H // NCORES   # heads per core = 2
COLS = 3 * HPC * D  # 384 qkv columns per core
SCALE = 1.0 / float(np.sqrt(D))

_CACHE = {}


def _build_program(dbg=False):
    import concourse.bass as bass  # noqa: F401
    import concourse.tile as tile
    from concourse import bacc, mybir

    f16 = mybir.dt.float16
    f32 = mybir.dt.float32
    Exp = mybir.ActivationFunctionType.Exp

    nc = bacc.Bacc("TRN2", target_bir_lowering=False, debug=False)

    if dbg:
        dbg_qkvT2 = nc.dram_tensor("dbg_qkvT2", [128, 3 * S], f16, kind="ExternalOutput")
        dbg_v2 = nc.dram_tensor("dbg_v2", [128, 160 * (S // 128)], f16, kind="ExternalOutput")
        dbg_attnT2 = nc.dram_tensor("dbg_attnT2", [128, S], f16, kind="ExternalOutput")
        dbg_rb = nc.dram_tensor("dbg_rb", [4, 64, 512], f32, kind="ExternalOutput")
        dbg_ex = nc.dram_tensor("dbg_ex", [4, 128, 1024], f16, kind="ExternalOutput")

    xT = nc.dram_tensor("xT", [B, E, S], f16, kind="ExternalInput")
    wqkv = nc.dram_tensor("wqkv", [E, COLS], f16, kind="ExternalInput")
    bqkv = nc.dram_tensor("bqkv", [128, 3], f32, kind="ExternalInput")
    wo2 = nc.dram_tensor("wo2", [16, 128, E], f16, kind="ExternalInput")
    bo2 = nc.dram_tensor("bo2", [128, E], f16, kind="ExternalInput")
    trimask = nc.dram_tensor("trimask", [128, 128], f16, kind="ExternalInput")
    out = nc.dram_tensor("out", [B, HPC, 128, E], f32, kind="ExternalOutput")

    with tile.TileContext(nc) as tc:
        with (
            tc.tile_pool(name="const", bufs=1) as cp,
            tc.tile_pool(name="sb", bufs=2) as sb,
            tc.tile_pool(name="sb3", bufs=3) as sb3,
            tc.tile_pool(name="ps", bufs=2, space="PSUM") as ps,
        ):
            # ---- constants resident in SBUF for the whole kernel ----
            wqkv_sb = cp.tile([128, 8 * COLS], f16)   # [p, ec*384+col]
            nc.sync.dma_start(
                wqkv_sb.rearrange("p (ec c) -> p ec c", ec=8),
                wqkv.ap().rearrange("(ec p) c -> p ec c", p=128),
            )
            bqkv_sb = cp.tile([128, 3], f32)
            nc.sync.dma_start(bqkv_sb, bqkv.ap())
            trimask_sb = cp.tile([128, 128], f16)
            nc.sync.dma_start(trimask_sb, trimask.ap())
            ones_sb = cp.tile([128, 128], f16)
            nc.vector.memset(ones_sb, 1.0)
            # o_proj weights are not needed until the first batch's o_proj;
            # load them on the ACT HWDGE ring so they don't block the SP ring
            wo2_sb = cp.tile([128, 16 * E], f16)      # [p, w*1024+c]
            nc.scalar.dma_start(
                wo2_sb.rearrange("p (w c) -> p w c", w=16),
                wo2.ap().rearrange("w p c -> p w c"),
            )
            bo2_sb = cp.tile([128, E], f16)
            nc.scalar.dma_start(bo2_sb, bo2.ap())

            for b in range(B):
                # ---- load x^T for this batch: [p, ec*2048+s] ----
                xt_sb = sb.tile([128, 8 * S], f16, tag="xt")
                xt_dram = xT.ap()[b].rearrange("(ec p) s -> p ec s", p=128)
                if b == 0:
                    # kernel warm-up: land the first matmul's rhs (ec0, first
                    # 512 cols) as its own small DMA so PE starts ~3us earlier
                    nc.sync.dma_start(xt_sb[:, 0:512], xt_dram[:, 0, 0:512])
                    nc.sync.dma_start(xt_sb[:, 512:S], xt_dram[:, 0, 512:S])
                    for ec in range(1, 8):
                        nc.sync.dma_start(
                            xt_sb[:, ec * S : (ec + 1) * S], xt_dram[:, ec]
                        )
                else:
                    for ec in range(8):
                        nc.sync.dma_start(
                            xt_sb[:, ec * S : (ec + 1) * S], xt_dram[:, ec]
                        )

                # ---- qkvT2 = wqkv^T @ x^T, head-major [col2, s] ----
                # col chunks: m=0 -> [q_h0|q_h1], m=1 -> [k_h0|k_h1], m=2 -> [v_h0|v_h1]
                qkvT2_sb = sb.tile([128, 3 * S], f16, tag="qkvT2")
                for m in range(3):
                    for n in range(S // 512):
                        pq = ps.tile([128, 512], f32, tag="acc", name="pq", bufs=4)
                        for ec in range(8):
                            nc.tensor.matmul(
                                pq,
                                wqkv_sb[:, ec * COLS + m * 128 : ec * COLS + (m + 1) * 128],
                                xt_sb[:, ec * S + n * 512 : ec * S + (n + 1) * 512],
                                start=(ec == 0),
                                stop=(ec == 7),
                            )
                        nc.vector.tensor_scalar_add(
                            qkvT2_sb[:, m * S + n * 512 : m * S + (n + 1) * 512],
                            pq,
                            bqkv_sb[:, m : m + 1],
                        )

                # ---- v2: transpose vT2 [d2, s] -> [s, d] per 128-chunk (xbar), ----
                # ---- augmented with a ones column per head for fused rowsums ----
                # chunk layout (stride 160): [v_h0(64) | ones | pad15 | v_h1(64) | ones | pad15]
                v2_sb = sb.tile([128, 160 * (S // 128)], f16, tag="v2")
                v2v = v2_sb.rearrange("p (c t) -> p c t", t=160)
                for st in range(S // 128):
                    for h in range(2):
                        nc.sync.dma_start(
                            v2_sb[:, st * 160 + h * 80 : st * 160 + h * 80 + 64],
                            qkvT2_sb[h * 64 : (h + 1) * 64,
                                     2 * S + st * 128 : 2 * S + (st + 1) * 128],
                            transpose=True,
                        )
                nc.gpsimd.memset(v2v[:, :, 64:65], 1.0)
                nc.gpsimd.memset(v2v[:, :, 144:145], 1.0)

                if dbg and b == 0:
                    nc.sync.dma_start(dbg_qkvT2.ap(), qkvT2_sb)
                    nc.sync.dma_start(dbg_v2.ap(), v2_sb)

                # ---- attention, 512-wide q chunks ----
                # attn (normalized, fp16): h0 -> partitions 0-63 of attn2_sb,
                # h1 staged on partitions 0-63 of attn1_tmp, then DMA-moved to
                # partitions 64-127 of attn2_sb for row-packed o_proj.
                attn2_sb = sb.tile([128, S], f16, tag="attn2", name="attn2_sb")
                attn1_tmp = sb.tile([64, S], f16, tag="attn1t", name="attn1_tmp")
                attn_sb = [attn2_sb, attn1_tmp]
                for gq in range(S // 512):
                    njk = 4 * gq + 4
                    # [65, 512]: rows 0-63 = sum exp*v (transposed), row 64 = rowsum
                    att_ps = [
                        ps.tile([65, 512], f32, tag="acc", name=f"att{h}_ps", bufs=4)
                        for h in range(2)
                    ]
                    for kj in range(njk):
                        q_lo = max(gq * 512, kj * 128)
                        W = gq * 512 + 512 - q_lo
                        qo = q_lo - gq * 512
                        sc_ps = ps.tile([128, 1024], f32, tag="scores", name="sc_ps")
                        ex_sb = sb3.tile([128, 1024], f16, tag="expT", name="ex_sb")
                        for h in range(2):
                            # scoresT[k, q] = (kT chunk)^T-contracted with qT
                            nc.tensor.matmul(
                                sc_ps[:, h * 512 + qo : h * 512 + qo + W],
                                qkvT2_sb[h * 64 : (h + 1) * 64,
                                         S + kj * 128 : S + (kj + 1) * 128],
                                qkvT2_sb[h * 64 : (h + 1) * 64, q_lo : q_lo + W],
                                start=True,
                                stop=True,
                                tile_position=(h * 64, 0),
                            )
                        # exp over both heads in one ACT instruction
                        nc.scalar.activation(
                            ex_sb.rearrange("p (h q) -> p h q", h=2)[:, :, qo : qo + W],
                            sc_ps.rearrange("p (h q) -> p h q", h=2)[:, :, qo : qo + W],
                            Exp,
                            scale=SCALE,
                        )
                        if kj >= 4 * gq:  # diagonal chunk: zero out k > q
                            for h in range(2):
                                nc.vector.tensor_mul(
                                    ex_sb[:, h * 512 + qo : h * 512 + qo + 128],
                                    ex_sb[:, h * 512 + qo : h * 512 + qo + 128],
                                    trimask_sb,
                                )
                        if dbg and b == 0 and kj == 0:
                            nc.sync.dma_start(dbg_ex.ap()[gq], ex_sb)
                        for h in range(2):
                            nc.tensor.matmul(
                                att_ps[h][:, qo : qo + W],
                                v2_sb[:, kj * 160 + h * 80 : kj * 160 + h * 80 + 65],
                                ex_sb[:, h * 512 + qo : h * 512 + qo + W],
                                start=(kj == 0),
                                stop=(kj == njk - 1),
                            )
                    # normalize this q-chunk
                    for h in range(2):
                        rr = sb.tile([1, 512], f32, tag=f"rr{h}", name=f"rr{h}")
                        nc.vector.reciprocal(rr, att_ps[h][64:65, :])
                        rb = sb.tile([64, 512], f32, tag=f"rb{h}", name=f"rb{h}")
                        nc.gpsimd.partition_broadcast(rb, rr)
                        nc.vector.tensor_mul(
                            attn_sb[h][0:64, gq * 512 : (gq + 1) * 512],
                            att_ps[h][0:64, :],
                            rb,
                        )
                        if dbg and b == 0 and h == 0:
                            nc.sync.dma_start(dbg_rb.ap()[gq], rb)
                    # move h1's attn rows to partitions 64-127 (row-packed o_proj)
                    nc.sync.dma_start(
                        attn2_sb[64:128, gq * 512 : (gq + 1) * 512],
                        attn1_tmp[:, gq * 512 : (gq + 1) * 512],
                    )

                if dbg and b == 0:
                    nc.sync.dma_start(dbg_attnT2.ap(), attn2_sb)

                # ---- o_proj: out_band[u, c] = sum_{w,d} attn[d, u*16+w] Wo[w*64+d, c] ----
                # two heads row-packed into PE row groups 0-1 / 2-3; head MMs
                # interleaved per w so disjoint row groups execute concurrently
                attv = attn2_sb.rearrange("p (u w) -> p w u", w=16)
                out_sbs = [
                    sb.tile([128, E], f32, tag=f"outsb{h}", name=f"out{h}_sb")
                    for h in range(2)
                ]
                for n2 in range(2):
                    po = [
                        ps.tile([128, 512], f32, tag="acc", name=f"po{h}", bufs=4)
                        for h in range(2)
                    ]
                    for w in range(16):
                        for h in range(2):
                            nc.tensor.matmul(
                                po[h],
                                attv[h * 64 : (h + 1) * 64, w : w + 1, :],
                                wo2_sb[h * 64 : (h + 1) * 64,
                                       w * E + n2 * 512 : w * E + (n2 + 1) * 512],
                                start=(w == 0),
                                stop=False,
                                tile_position=(h * 64, 0),
                            )
                    for h in range(2):
                        # bias row via K=1 ones matmul
                        nc.tensor.matmul(
                            po[h],
                            ones_sb[h * 64 : h * 64 + 1, :],
                            bo2_sb[h * 64 : h * 64 + 1, n2 * 512 : (n2 + 1) * 512],
                            start=False,
                            stop=True,
                            tile_position=(h * 64, 0),
                        )
                        nc.vector.tensor_copy(
                            out_sbs[h][:, n2 * 512 : (n2 + 1) * 512], po[h]
                        )
                for h in range(2):
                    nc.scalar.dma_start(out.ap()[b, h], out_sbs[h])

    nc.compile()
    return nc


def _get_program(dbg=False):
    key = ("nc", dbg)
    if key not in _CACHE:
        _CACHE[key] = _build_program(dbg)
    return _CACHE[key]


def _host_inputs(x, Wqkv, bqkv, Wo, bo):
    """Build per-core input maps (host-side layout prep: cast/slice/transpose)."""
    xT = np.ascontiguousarray(x.transpose(0, 2, 1)).astype(np.float16)

    wo16 = Wo.astype(np.float16)
    wo2 = np.empty((16, 128, E), np.float16)
    for w in range(16):
        wo2[w, 0:64] = wo16[w * 64 : (w + 1) * 64]
        wo2[w, 64:128] = wo16[w * 64 : (w + 1) * 64]

    bo2 = np.zeros((128, E), np.float16)
    bo2[0] = bo.astype(np.float16)
    bo2[64] = bo.astype(np.float16)

    k_idx = np.arange(128)[:, None]
    q_idx = np.arange(128)[None, :]
    trimask = (k_idx <= q_idx).astype(np.float16)

    in_maps = []
    for c in range(NCORES):
        cols = []
        for off in (0, 64, 128):  # q, k, v
            for h in (HPC * c, HPC * c + 1):
                cols.extend(range(h * 3 * D + off, h * 3 * D + off + 64))
        cols = np.asarray(cols)
        in_maps.append(
            {
                "xT": xT,
                "wqkv": np.ascontiguousarray(Wqkv[:, cols]).astype(np.float16),
                "bqkv": np.ascontiguousarray(
                    bqkv[cols].reshape(3, 128).T
                ).astype(np.float32),
                "wo2": wo2,
                "bo2": bo2,
                "trimask": trimask,
            }
        )
    return in_maps


def kernel(x, mask, Wqkv, bqkv, Wo, bo, _n_cores=NCORES, _trace=False, _dbg=False):
    """Full-input, full-output MHA. `mask` is the causal tril mask (hardcoded)."""
    from concourse.bass_utils import run_bass_kernel_spmd

    nc = _get_program(_dbg)
    in_maps = _host_inputs(
        np.asarray(x), np.asarray(Wqkv), np.asarray(bqkv), np.asarray(Wo), np.asarray(bo)
    )[:_n_cores]
    res = run_bass_kernel_spmd(
        nc, in_maps, core_ids=list(range(_n_cores)), trace=_trace
    )
    out_full = np.zeros((B, S, E), np.float32)
    for c in range(_n_cores):
        o = res.results[c]["out"]  # [B, HPC, 128, E]
        for h in range(HPC):
            g = HPC * c + h
            out_full[:, g * 128 : (g + 1) * 128, :] = o[:, h]
    _CACHE["last_results"] = res
    return out_full


def time_kernel(x, Wqkv, bqkv, Wo, bo, n_iters=20, n_cores=NCORES):
    """Time repeated on-device executions with device-resident inputs.

    Returns (best_ns, mean_ns) per execution of the full 8-core SPMD launch.
    """
    import time

    import jax
    import numpy as _np
    from jax.sharding import Mesh, PartitionSpec
    from jax.experimental.shard_map import shard_map
    from concourse import bass2jax, mybir

    nc = _get_program()
    bass2jax.install_neuronx_cc_hook()

    in_maps = _host_inputs(x, Wqkv, bqkv, Wo, bo)[:n_cores]

    partition_name = nc.partition_id_tensor.name if nc.partition_id_tensor else None
    in_names, out_names, out_avals, zero_outs = [], [], [], []
    for alloc in nc.m.functions[0].allocations:
        if not isinstance(alloc, mybir.MemoryLocationSet):
            continue
        name = alloc.memorylocations[0].name
        if alloc.kind == "ExternalInput":
            if name != partition_name:
                in_names.append(name)
        elif alloc.kind == "ExternalOutput":
            out_names.append(name)
            shape = tuple(alloc.tensor_shape)
            dtype = mybir.dt.np(alloc.dtype)
            out_avals.append(jax.core.ShapedArray(shape, dtype))
            zero_outs.append(_np.zeros(shape, dtype))
    n_params = len(in_names)

    def _body(*args):
        operands = list(args)
        all_names = in_names + out_names
        if partition_name is not None:
            operands.append(bass2jax.partition_id_tensor())
            all_names = all_names + [partition_name]
        outs = bass2jax._bass_exec_p.bind(
            *operands,
            out_avals=tuple(out_avals),
            in_names=tuple(all_names),
            out_names=tuple(out_names),
            lowering_input_output_aliases=(),
            sim_require_finite=True,
            sim_require_nnan=True,
            nc=nc,
        )
        return tuple(outs)

    devices = jax.devices()[:n_cores]
    mesh = Mesh(_np.asarray(devices), ("core",))
    nin = n_params + len(out_names)
    fn = jax.jit(
        shard_map(
            _body,
            mesh=mesh,
            in_specs=(PartitionSpec("core"),) * nin,
            out_specs=(PartitionSpec("core"),) * len(out_names),
            check_rep=False,
        ),
        keep_unused=True,
    )
    concat_in = [
        _np.concatenate([in_maps[c][nm] for c in range(n_cores)], axis=0)
        for nm in in_names
    ] + [_np.zeros((n_cores * z.shape[0], *z.shape[1:]), z.dtype) for z in zero_outs]
    from jax.sharding import NamedSharding

    sharding = NamedSharding(mesh, PartitionSpec("core"))
    dev_in = [jax.device_put(a, sharding) for a in concat_in]

    # warmup/compile
    outs = fn(*dev_in)
    jax.block_until_ready(outs)
    times = []
    for _ in range(n_iters):
        t0 = time.perf_counter()
        outs = fn(*dev_in)
        jax.block_until_ready(outs)
        times.append((time.perf_counter() - t0) * 1e9)
    return min(times), sum(times) / len(times)



# revision 55
# speedup vs baseline: 1.2512x; 1.2512x over previous
"""Trainium2 Bass kernel for nn_MultiHeadAttention_46093589021200.

Causal MHA: B=4, S=2048, E=1024, H=16, D=64, with the reference's
"no-transpose-back" reshape (b,h,s,d)->(b,s,e) before the output projection.

Sharding: pure head-parallel, 2 heads per core, zero collectives.
Because of the reshape quirk, output rows s' in [h*128,(h+1)*128) depend only
on head h, so each core produces two independent 128-row output bands per
batch.

Device algorithm (per core, fp16 compute / fp32 PSUM accumulation):
  - qkvT = Wqkv_c^T @ x^T computed directly in head-major [col, s] layout
    (x is passed pre-transposed+pre-cast from the host; contraction over e
    in 8 PSUM-accumulated K=128 chunks).
  - v transposed to [s, d] via the DMA xbar (one batched transpose per
    (head, 512-col chunk)), augmented with a ones column per head so the PV
    matmul also produces softmax denominators (M=65).
  - scoresT[k,q] per 128-k chunk on PE, two heads packed into row groups
    0-1 / 2-3 of the systolic array (K=64 each).
  - exp on ACT, one instruction covering both heads per chunk
    (scale=1/sqrt(D) folded in); causality = skipping k>q chunks entirely
    plus a triangular fp16 mask multiply on diagonal chunks.
  - PV accumulates att_aug[d+1, q] in PSUM with v_aug stationary; DVE
    reciprocal of the rowsum row + GPSIMD partition_broadcast + one DVE
    multiply produce normalized fp16 attn on partitions 0-63 of attB[h];
    a shifted SBUF->SBUF DMA copies attn (columns +1) onto partitions
    64-127 so o_proj can contract K=128 per matmul.
  - o_proj: per 128-row band, 8 K=128-chunk matmuls (w-pairs packed via the
    shifted copy) + a K=1 ones-row matmul for the bias.  o_proj for batch b
    is emitted after batch b+1's qkv so the normalize tail never stalls PE.

NOTE: column-positioned matmuls (tile_position=(0,32j), PSUM output at a
partition offset) mis-execute on this hardware path even though CoreSim
accepts them — this kernel uses row-group packing only.
"""

import sys

if "/opt/trn_rl_repo" not in sys.path:
    sys.path.insert(0, "/opt/trn_rl_repo")

import numpy as np

B, S, E, H = 4, 2048, 1024, 16
D = E // H          # 64
NCORES = 8
HPC = H // NCORES   # heads per core = 2
COLS = 3 * HPC * D  # 384 qkv columns per core
SCALE = 1.0 / float(np.sqrt(D))

_CACHE = {}


def _build_program():
    import concourse.bass as bass  # noqa: F401
    import concourse.tile as tile
    from concourse import bacc, mybir

    f16 = mybir.dt.float16
    f32 = mybir.dt.float32
    Exp = mybir.ActivationFunctionType.Exp

    nc = bacc.Bacc("TRN2", target_bir_lowering=False, debug=False)

    xT = nc.dram_tensor("xT", [B, E, S], f16, kind="ExternalInput")
    wqkv = nc.dram_tensor("wqkv", [E, COLS], f16, kind="ExternalInput")
    bqkv = nc.dram_tensor("bqkv", [128, 3], f32, kind="ExternalInput")
    wo8 = nc.dram_tensor("wo8", [8, 128, E], f16, kind="ExternalInput")
    bo1 = nc.dram_tensor("bo1", [1, E], f16, kind="ExternalInput")
    trimask = nc.dram_tensor("trimask", [128, 256], f16, kind="ExternalInput")
    out = nc.dram_tensor("out", [B, HPC, 128, E], f16, kind="ExternalOutput")

    with tile.TileContext(nc) as tc:
        with (
            tc.tile_pool(name="const", bufs=1) as cp,
            tc.tile_pool(name="sb", bufs=2) as sb,
            tc.tile_pool(name="sb3", bufs=3) as sb3,
            tc.tile_pool(name="ps", bufs=2, space="PSUM") as ps,
        ):
            # ---- constants resident in SBUF for the whole kernel ----
            # critical path (SP ring): wqkv, then batch-0 x slices
            wqkv_sb = cp.tile([128, 8 * COLS], f16)   # [p, ec*384+col]
            nc.sync.dma_start(
                wqkv_sb.rearrange("p (ec c) -> p ec c", ec=8),
                wqkv.ap().rearrange("(ec p) c -> p ec c", p=128),
            )
            # non-critical constants on the ACT HWDGE ring
            bqkv_sb = cp.tile([128, 3], f32)
            nc.scalar.dma_start(bqkv_sb, bqkv.ap())
            trimask_sb = cp.tile([128, 256], f16)  # two copies side by side
            nc.scalar.dma_start(trimask_sb, trimask.ap())
            bo_sb = cp.tile([1, E], f16)
            nc.scalar.dma_start(bo_sb, bo1.ap())
            # wo8 loads in per-j chunks, interleaved into batch 0's stream:
            # a long const DMA in flight would stall the transpose-vs-DMA
            # serialization guard ahead of the v2 transposes
            wo8_sb = cp.tile([128, 8 * E], f16)       # [p, j*1024+c]

            def emit_wo8(j0, j1):
                for j in range(j0, j1):
                    nc.scalar.dma_start(
                        wo8_sb[:, j * E : (j + 1) * E], wo8.ap()[j]
                    )

            ones_sb = cp.tile([1, 128], f16)
            nc.vector.memset(ones_sb, 1.0)

            def emit_xload(b, lo=0, hi=4, xt_sb=None):
                # b==0: n-chunk slices, staggered around vt0 so the
                # transpose guard only waits on slice 0
                if xt_sb is None:
                    xt_sb = sb.tile([128, 8 * S], f16, tag="xt")
                xt3 = xt_sb.rearrange("p (ec s) -> p ec s", ec=8)
                xd3 = xT.ap()[b].rearrange("(ec p) s -> p ec s", p=128)
                # on the SWDGE (gpsimd) ring: HWDGE x-load transfers would
                # make the transpose-serialization guard stall v2 transposes
                if b == 0:
                    for n in range(lo, hi):
                        nc.gpsimd.dma_start(
                            xt3[:, :, n * 512 : (n + 1) * 512],
                            xd3[:, :, n * 512 : (n + 1) * 512],
                        )
                else:
                    for ec in range(8):
                        nc.gpsimd.dma_start(xt3[:, ec], xd3[:, ec])
                return xt_sb

            def emit_qkv_mgroup(xt_sb, qkvT2_sb, n, m):
                # one 512-wide s-chunk of one of q/k/v:
                # m=0 -> [q_h0|q_h1], m=1 -> [k_h0|k_h1], m=2 -> v
                pq = ps.tile([128, 512], f32, tag="acc", name="pq", bufs=4)
                for ec in range(8):
                    nc.tensor.matmul(
                        pq,
                        wqkv_sb[:, ec * COLS + m * 128
                                : ec * COLS + (m + 1) * 128],
                        xt_sb[:, ec * S + n * 512 : ec * S + (n + 1) * 512],
                        start=(ec == 0),
                        stop=(ec == 7),
                    )
                # bias-add + PSUM->SBUF drain.  GPSIMD cannot read PSUM on
                # hardware, so split q/k (DVE) and v (ACT Copy-with-bias) to
                # keep the DVE queue short for the normalize chain.
                dst = qkvT2_sb[:, m * S + n * 512 : m * S + (n + 1) * 512]
                if m == 2:
                    nc.scalar.activation(
                        dst, pq, mybir.ActivationFunctionType.Identity,
                        bias=bqkv_sb[:, m : m + 1],
                    )
                else:
                    nc.vector.tensor_scalar_add(dst, pq, bqkv_sb[:, m : m + 1])

            def emit_vtrans(qkvT2_sb, v2v, n):
                # transpose vT2 [d2, s] -> [s, d] batched per (h, n-chunk)
                for h in range(2):
                    nc.sync.dma_start(
                        v2v[:, 4 * n : 4 * n + 4, h * 80 : h * 80 + 64],
                        qkvT2_sb[h * 64 : (h + 1) * 64,
                                 2 * S + n * 512 : 2 * S + (n + 1) * 512],
                        transpose=True,
                    )

            def emit_attn_gq(qkvT2_sb, v2_sb, attQ_sb, attT_sb, attB, gq,
                             fillers=()):
                fillers = dict(fillers)
                njk = 4 * gq + 4
                # q-major PV accumulators: [q=128, sub*65 + (d|rowsum)],
                # one per head.  N=65 per PV matmul (vs W) halves the PE
                # streaming cost; rowsums land per-partition so the
                # normalize is reciprocal + tensor_scalar (no broadcast).
                att_ps = [
                    ps.tile([128, 260], f32, tag="acc",
                            name=f"att{h}_ps", bufs=4)
                    for h in range(2)
                ]
                for kj in range(njk):
                    if kj in fillers:
                        fillers[kj]()
                    q_lo = max(gq * 512, kj * 128)
                    W = gq * 512 + 512 - q_lo
                    qo = q_lo - gq * 512
                    sc_ps = ps.tile([128, 1024], f32, tag="scores",
                                    name="sc_ps")
                    ex_sb = sb3.tile([128, 1024], f16, tag="expT",
                                     name="ex_sb")
                    for h in range(2):
                        # scoresT[k, q] = kT-chunk contracted with qT
                        nc.tensor.matmul(
                            sc_ps[:, h * 512 + qo : h * 512 + qo + W],
                            qkvT2_sb[h * 64 : (h + 1) * 64,
                                     S + kj * 128 : S + (kj + 1) * 128],
                            qkvT2_sb[h * 64 : (h + 1) * 64, q_lo : q_lo + W],
                            start=True,
                            stop=True,
                            tile_position=(h * 64, 0),
                        )
                    # exp over both heads in one ACT instruction
                    nc.scalar.activation(
                        ex_sb.rearrange("p (h q) -> p h q", h=2)
                        [:, :, qo : qo + W],
                        sc_ps.rearrange("p (h q) -> p h q", h=2)
                        [:, :, qo : qo + W],
                        Exp,
                        scale=SCALE,
                    )
                    if kj >= 4 * gq:  # diagonal chunk: zero out k > q
                        exv = ex_sb.rearrange("p (h q) -> p h q", h=2)
                        nc.vector.tensor_mul(
                            exv[:, :, qo : qo + 128],
                            exv[:, :, qo : qo + 128],
                            trimask_sb.rearrange("p (h q) -> p h q", h=2),
                        )
                    # att_q[q, d_aug] += ex[k, q]^T-contracted with v_aug
                    # (ex chunk is the stationary; LDWEIGHTS overlaps).
                    # One accumulation group per tile: PSUM groups are
                    # per-bank, so the sub-chunk column slices share a
                    # single start/stop window.
                    for h in range(2):
                        for c in range(max(0, kj - 4 * gq), 4):
                            nc.tensor.matmul(
                                att_ps[h][:, c * 65 : c * 65 + 65],
                                ex_sb[:, h * 512 + c * 128
                                      : h * 512 + (c + 1) * 128],
                                v2_sb[:, kj * 160 + h * 80
                                      : kj * 160 + h * 80 + 65],
                                start=(kj == 0 and c == 0),
                                stop=(kj == njk - 1 and c == 3),
                            )
                # normalize: stage PSUM->SBUF in one copy (frees the 'acc'
                # ring, which gates the next qkv groups, ~1 us sooner), then
                # per-partition reciprocal + scalar muls into attQ
                # [q, c*128 + h*64 + d] fp16
                for h in range(2):
                    st = sb.tile([128, 260], f32, tag=f"st{h}",
                                 name=f"st{h}")
                    nc.vector.tensor_copy(st, att_ps[h])
                    rr4 = sb.tile([128, 4], f32, tag=f"rr{h}", name=f"rr{h}")
                    nc.vector.reciprocal(
                        rr4, st.rearrange("p (c t) -> p c t", t=65)[:, :, 64]
                    )
                    for c in range(4):
                        nc.vector.tensor_scalar_mul(
                            attQ_sb[:, gq * 512 + c * 128 + h * 64
                                    : gq * 512 + c * 128 + h * 64 + 64],
                            st[:, c * 65 : c * 65 + 64],
                            rr4[:, c : c + 1],
                        )
                # (the attQ->attT transpose and attB copies are emitted
                # later via emit_attn_finish, so they never park ahead of
                # the next v2 transpose on the SP ring)

            def emit_attn_finish(attQ_sb, attT_sb, attB, gq):
                # transpose attQ block -> attT [h*64+d, q] via the DMA xbar
                attT3 = attT_sb.rearrange("p (c t) -> p c t", t=128)
                nc.sync.dma_start(
                    attT3[:, 4 * gq : 4 * gq + 4, :],
                    attQ_sb[:, gq * 512 : (gq + 1) * 512],
                    transpose=True,
                )
                # o_proj operand (attB cols h*S+q): top = attn_h[d, q],
                # bottom = attn_h[d, q+1] (shifted), so K=128 w-pair chunks
                # read with one stride-16 AP.  Block gq enables shifted-dest
                # columns [gq*512-1, gq*512+511).
                lo = gq * 512
                src_lo = max(1, lo)
                for h in range(2):
                    nc.sync.dma_start(
                        attB[0:64, h * S + lo : h * S + lo + 512],
                        attT_sb[h * 64 : (h + 1) * 64, lo : lo + 512],
                    )
                    nc.sync.dma_start(
                        attB[64:128, h * S + src_lo - 1 : h * S + lo + 511],
                        attT_sb[h * 64 : (h + 1) * 64, src_lo : lo + 512],
                    )

            def emit_oproj_group(b, attB, out_sbs, h, n2):
                # out_band[u, c] = sum_j sum_{k<128}
                #   attB[k, h*S + u*16+2j] Wo[128j+k, c]   (K=128 per matmul)
                attv = attB[:, h * S : (h + 1) * S].rearrange(
                    "p (u w) -> p w u", w=16)
                po = ps.tile([128, 512], f32, tag="acc", name="po", bufs=4)
                for j in range(8):
                    nc.tensor.matmul(
                        po,
                        attv[:, 2 * j, :],
                        wo8_sb[:, j * E + n2 * 512 : j * E + n2 * 512 + 512],
                        start=(j == 0),
                        stop=False,
                    )
                # bias row via K=1 ones matmul
                nc.tensor.matmul(
                    po,
                    ones_sb[0:1, :],
                    bo_sb[0:1, n2 * 512 : (n2 + 1) * 512],
                    start=False,
                    stop=True,
                )
                nc.vector.tensor_copy(
                    out_sbs[h][:, n2 * 512 : (n2 + 1) * 512], po
                )
                if n2 == 1:
                    nc.sync.dma_start(out.ap()[b, h], out_sbs[h])

            # schedule per batch: [qkv n0, qkv n1, gq0, qkv n2, gq1, qkv n3,
            # gq2, gq3] with o_proj(b-1) groups woven into the ACT-bound
            # gq2/gq3 regions so PE never drains while ACT catches up.
            prev = None
            fin_args = None
            for b in range(B):
                xt_sb = emit_xload(b, 0, 1)
                qkvT2_sb = sb.tile([128, 3 * S], f16, tag="qkvT2")
                # v2 chunk layout (stride 160):
                #   [v_h0(64) | ones | pad15 | v_h1(64) | ones | pad15]
                v2_sb = sb.tile([128, 160 * (S // 128)], f16, tag="v2")
                v2v = v2_sb.rearrange("p (c t) -> p c t", t=160)
                nc.gpsimd.memset(v2v[:, :, 64:65], 1.0)
                nc.gpsimd.memset(v2v[:, :, 144:145], 1.0)
                attQ_sb = sb.tile([128, S], f16, tag="attQ", name="attQ")
                attT_sb = sb.tile([128, S], f16, tag="attT", name="attT")
                attB = sb.tile([128, 2 * S], f16, tag="attB", name="attB")

                def qkv_m(n, m, vt=False):
                    def fn():
                        emit_qkv_mgroup(xt_sb, qkvT2_sb, n, m)
                        if vt:
                            emit_vtrans(qkvT2_sb, v2v, n)
                    return fn

                def attn(gq, fillers=()):
                    emit_attn_gq(qkvT2_sb, v2_sb, attQ_sb, attT_sb, attB,
                                 gq, fillers)

                def finish(gq):
                    def fn():
                        emit_attn_finish(attQ_sb, attT_sb, attB, gq)
                    return fn

                # v-group first so the v2 transpose (2.2 us DMA latency)
                # overlaps the q/k groups instead of stalling gq0's PV
                emit_qkv_mgroup(xt_sb, qkvT2_sb, 0, 2)
                emit_vtrans(qkvT2_sb, v2v, 0)
                if b == 0:
                    emit_xload(0, 1, 4, xt_sb)  # rest of batch-0 x after vt0
                emit_qkv_mgroup(xt_sb, qkvT2_sb, 0, 0)
                emit_qkv_mgroup(xt_sb, qkvT2_sb, 0, 1)
                emit_qkv_mgroup(xt_sb, qkvT2_sb, 1, 0)
                emit_qkv_mgroup(xt_sb, qkvT2_sb, 1, 1)
                attn(0, [(1, qkv_m(1, 2, vt=True))])
                if b == 0:
                    emit_wo8(0, 8)  # ACT ring; emitted after batch 0's
                    # first attention block so the transfers never sit in
                    # flight ahead of the startup v2 transposes
                # the deferred gq3-finish of b-1 goes after vt2 so its
                # copies never park ahead of v2 transposes on the SP ring
                f1 = [(1, qkv_m(2, 0)), (3, qkv_m(2, 1)),
                      (5, qkv_m(2, 2, vt=True)), (7, finish(0))]
                if fin_args is not None:
                    fa = fin_args
                    f1.insert(3, (6, lambda: emit_attn_finish(*fa, 3)))
                attn(1, f1)
                # n3's k/v groups defer into gq3 (k chunks 12-15 are first
                # touched at kj=12), freeing gq2 slots for o_proj(b-1);
                # the 'acc' PSUM ring (4 bufs) fixes which po groups may
                # precede the att_ps allocations
                if prev is not None:
                    pb, pattB, pout = prev
                    attn(2, [(1, qkv_m(3, 0)), (3, finish(1)),
                             (4, lambda: emit_oproj_group(
                                 pb, pattB, pout, 0, 0)),
                             (8, lambda: emit_oproj_group(
                                 pb, pattB, pout, 0, 1))])
                    attn(3, [(1, qkv_m(3, 1)), (4, qkv_m(3, 2, vt=True)),
                             (6, finish(2))])
                    emit_oproj_group(pb, pattB, pout, 1, 0)
                    emit_oproj_group(pb, pattB, pout, 1, 1)
                else:
                    attn(2, [(1, qkv_m(3, 0)), (3, finish(1))])
                    attn(3, [(1, qkv_m(3, 1)), (4, qkv_m(3, 2, vt=True)),
                             (6, finish(2))])
                out_sbs = [
                    sb.tile([128, E], f16, tag=f"outsb{h}", name=f"out{h}_sb")
                    for h in range(2)
                ]
                prev = (b, attB, out_sbs)
                fin_args = (attQ_sb, attT_sb, attB)
            pb, pattB, pout = prev
            emit_attn_finish(*fin_args, 3)
            for h in range(2):
                for n2 in range(2):
                    emit_oproj_group(pb, pattB, pout, h, n2)

    nc.compile()
    return nc


def _get_program(dbg=False):
    key = ("nc",)
    if key not in _CACHE:
        _CACHE[key] = _build_program()
    return _CACHE[key]


def _host_inputs(x, Wqkv, bqkv, Wo, bo):
    """Build per-core input maps (host-side layout prep: cast/slice/transpose)."""
    xT = np.ascontiguousarray(x.transpose(0, 2, 1)).astype(np.float16)

    wo8 = np.ascontiguousarray(
        Wo.astype(np.float16).reshape(8, 128, E)
    )
    bo1 = bo.astype(np.float16)[None, :]

    k_idx = np.arange(128)[:, None]
    q_idx = np.arange(128)[None, :]
    tri = (k_idx <= q_idx).astype(np.float16)
    trimask = np.concatenate([tri, tri], axis=1)  # one copy per head

    in_maps = []
    for c in range(NCORES):
        cols = []
        for off in (0, 64, 128):  # q, k, v
            for h in (HPC * c, HPC * c + 1):
                cols.extend(range(h * 3 * D + off, h * 3 * D + off + 64))
        cols = np.asarray(cols)
        in_maps.append(
            {
                "xT": xT,
                "wqkv": np.ascontiguousarray(Wqkv[:, cols]).astype(np.float16),
                "bqkv": np.ascontiguousarray(
                    bqkv[cols].reshape(3, 128).T
                ).astype(np.float32),
                "wo8": wo8,
                "bo1": bo1,
                "trimask": trimask,
            }
        )
    return in_maps


def kernel(x, mask, Wqkv, bqkv, Wo, bo, _n_cores=NCORES, _trace=False, _dbg=False):
    """Full-input, full-output MHA. `mask` is the causal tril mask (hardcoded)."""
    from concourse.bass_utils import run_bass_kernel_spmd

    nc = _get_program()
    in_maps = _host_inputs(
        np.asarray(x), np.asarray(Wqkv), np.asarray(bqkv), np.asarray(Wo), np.asarray(bo)
    )[:_n_cores]
    res = run_bass_kernel_spmd(
        nc, in_maps, core_ids=list(range(_n_cores)), trace=_trace
    )
    out_full = np.zeros((B, S, E), np.float32)
    for c in range(_n_cores):
        o = res.results[c]["out"]  # [B, HPC, 128, E]
        for h in range(HPC):
            g = HPC * c + h
            out_full[:, g * 128 : (g + 1) * 128, :] = o[:, h]
    _CACHE["last_results"] = res
    return out_full


# revision 64
# speedup vs baseline: 1.3444x; 1.0745x over previous
"""Trainium2 Bass kernel for nn_MultiHeadAttention_46093589021200.

Causal MHA: B=4, S=2048, E=1024, H=16, D=64, with the reference's
"no-transpose-back" reshape (b,h,s,d)->(b,s,e) before the output projection.

Sharding: pure head-parallel, 2 heads per core, zero collectives.
Because of the reshape quirk, output rows s' in [h*128,(h+1)*128) depend only
on head h, so each core produces two independent 128-row output bands per
batch.

Device algorithm (per core, fp16 compute / fp32 PSUM accumulation):
  - qkvT = Wqkv_c^T @ x^T computed directly in head-major [col, s] layout
    (x is passed pre-transposed+pre-cast from the host; contraction over e
    in 8 PSUM-accumulated K=128 chunks).
  - v transposed to [s, d] via the DMA xbar (one batched transpose per
    (head, 512-col chunk)), augmented with a ones column per head so the PV
    matmul also produces softmax denominators (M=65).
  - scoresT[k,q] per 128-k chunk on PE, two heads packed into row groups
    0-1 / 2-3 of the systolic array (K=64 each).
  - exp on ACT, one instruction covering both heads per chunk
    (scale=1/sqrt(D) folded in); causality = skipping k>q chunks entirely
    plus a triangular fp16 mask multiply on diagonal chunks.
  - PV accumulates att_aug[d+1, q] in PSUM with v_aug stationary; DVE
    reciprocal of the rowsum row + GPSIMD partition_broadcast + one DVE
    multiply produce normalized fp16 attn on partitions 0-63 of attB[h];
    a shifted SBUF->SBUF DMA copies attn (columns +1) onto partitions
    64-127 so o_proj can contract K=128 per matmul.
  - o_proj: per 128-row band, 8 K=128-chunk matmuls (w-pairs packed via the
    shifted copy) + a K=1 ones-row matmul for the bias.  o_proj for batch b
    is emitted after batch b+1's qkv so the normalize tail never stalls PE.

NOTE: column-positioned matmuls (tile_position=(0,32j), PSUM output at a
partition offset) mis-execute on this hardware path even though CoreSim
accepts them — this kernel uses row-group packing only.
"""

import sys

if "/opt/trn_rl_repo" not in sys.path:
    sys.path.insert(0, "/opt/trn_rl_repo")

import numpy as np

B, S, E, H = 4, 2048, 1024, 16
D = E // H          # 64
NCORES = 8
HPC = H // NCORES   # heads per core = 2
COLS = 3 * HPC * D  # 384 qkv columns per core
SCALE = 1.0 / float(np.sqrt(D))

_CACHE = {}


def _build_program():
    import concourse.bass as bass  # noqa: F401
    import concourse.tile as tile
    from concourse import bacc, mybir
    from concourse.instruction_name_ordered_set import InstructionNameOrderedSet

    f16 = mybir.dt.float16
    f32 = mybir.dt.float32
    Exp = mybir.ActivationFunctionType.Exp

    nc = bacc.Bacc("TRN2", target_bir_lowering=False, debug=False)

    xT = nc.dram_tensor("xT", [B, E, S], f16, kind="ExternalInput")
    wqkv = nc.dram_tensor("wqkv", [E, COLS], f16, kind="ExternalInput")
    bqkv = nc.dram_tensor("bqkv", [128, 3], f32, kind="ExternalInput")
    wo8 = nc.dram_tensor("wo8", [8, 128, E], f16, kind="ExternalInput")
    bo1 = nc.dram_tensor("bo1", [1, E], f16, kind="ExternalInput")
    trimask = nc.dram_tensor("trimask", [128, 256], f16, kind="ExternalInput")
    out = nc.dram_tensor("out", [B, HPC, 128, E], f16, kind="ExternalOutput")

    with tile.TileContext(nc) as tc:
        with (
            tc.tile_pool(name="const", bufs=1) as cp,
            tc.tile_pool(name="sb", bufs=2) as sb,
            tc.tile_pool(name="sb3", bufs=3) as sb3,
            tc.tile_pool(name="ps", bufs=2, space="PSUM") as ps,
        ):
            # ---- constants resident in SBUF for the whole kernel ----
            # critical path (SP ring): wqkv, then batch-0 x slices
            wqkv_sb = cp.tile([128, 8 * COLS], f16)   # [p, ec*384+col]
            nc.sync.dma_start(
                wqkv_sb.rearrange("p (ec c) -> p ec c", ec=8),
                wqkv.ap().rearrange("(ec p) c -> p ec c", p=128),
            )
            # non-critical constants on the ACT HWDGE ring
            bqkv_sb = cp.tile([128, 3], f32)
            nc.scalar.dma_start(bqkv_sb, bqkv.ap())
            trimask_sb = cp.tile([128, 256], f16)  # two copies side by side
            nc.scalar.dma_start(trimask_sb, trimask.ap())
            bo_sb = cp.tile([1, E], f16)
            nc.scalar.dma_start(bo_sb, bo1.ap())
            # wo8 loads in per-j chunks, interleaved into batch 0's stream:
            # a long const DMA in flight would stall the transpose-vs-DMA
            # serialization guard ahead of the v2 transposes
            wo8_sb = cp.tile([128, 8 * E], f16)       # [p, j*1024+c]

            def emit_wo8(j0, j1):
                for j in range(j0, j1):
                    nc.scalar.dma_start(
                        wo8_sb[:, j * E : (j + 1) * E], wo8.ap()[j]
                    )

            ones_sb = cp.tile([1, 128], f16)
            nc.vector.memset(ones_sb, 1.0)

            def emit_xload(b, lo=0, hi=4, xt_sb=None):
                # b==0: n-chunk slices, staggered around vt0 so the
                # transpose guard only waits on slice 0
                if xt_sb is None:
                    xt_sb = sb.tile([128, 8 * S], f16, tag="xt")
                xt3 = xt_sb.rearrange("p (ec s) -> p ec s", ec=8)
                xd3 = xT.ap()[b].rearrange("(ec p) s -> p ec s", p=128)
                # on the SWDGE (gpsimd) ring: HWDGE x-load transfers would
                # make the transpose-serialization guard stall v2 transposes
                if b == 0:
                    for n in range(lo, hi):
                        nc.gpsimd.dma_start(
                            xt3[:, :, n * 512 : (n + 1) * 512],
                            xd3[:, :, n * 512 : (n + 1) * 512],
                        )
                else:
                    for ec in range(8):
                        nc.gpsimd.dma_start(xt3[:, ec], xd3[:, ec])
                return xt_sb

            def emit_qkv_mgroup(xt_sb, qkvT2_sb, n, m):
                # one 512-wide s-chunk of one of q/k/v:
                # m=0 -> [q_h0|q_h1], m=1 -> [k_h0|k_h1], m=2 -> v
                pq = ps.tile([128, 512], f32, tag="acc", name="pq", bufs=4)
                for ec in range(8):
                    nc.tensor.matmul(
                        pq,
                        wqkv_sb[:, ec * COLS + m * 128
                                : ec * COLS + (m + 1) * 128],
                        xt_sb[:, ec * S + n * 512 : ec * S + (n + 1) * 512],
                        start=(ec == 0),
                        stop=(ec == 7),
                    )
                # bias-add + PSUM->SBUF drain.  GPSIMD cannot read PSUM on
                # hardware, so split q/k (DVE) and v (ACT Copy-with-bias) to
                # keep the DVE queue short for the normalize chain.
                dst = qkvT2_sb[:, m * S + n * 512 : m * S + (n + 1) * 512]
                if m == 2:
                    nc.scalar.activation(
                        dst, pq, mybir.ActivationFunctionType.Identity,
                        bias=bqkv_sb[:, m : m + 1],
                    )
                else:
                    nc.vector.tensor_scalar_add(dst, pq, bqkv_sb[:, m : m + 1])

            def emit_vtrans(qkvT2_sb, v2v, n):
                # transpose vT2 [d2, s] -> [s, d] batched per (h, n-chunk)
                insts = []
                for h in range(2):
                    insts.append(nc.sync.dma_start(
                        v2v[:, 4 * n : 4 * n + 4, h * 80 : h * 80 + 64],
                        qkvT2_sb[h * 64 : (h + 1) * 64,
                                 2 * S + n * 512 : 2 * S + (n + 1) * 512],
                        transpose=True,
                    ))
                return insts

            def emit_attn_gq(qkvT2_sb, v2_sb, attQ_sb, attT_sb, attB, gq,
                             fillers=()):
                fillers = dict(fillers)
                njk = 4 * gq + 4
                # q-major PV accumulators: [q=128, sub*65 + (d|rowsum)],
                # one per head.  N=65 per PV matmul (vs W) halves the PE
                # streaming cost; rowsums land per-partition so the
                # normalize is reciprocal + tensor_scalar (no broadcast).
                att_ps = [
                    ps.tile([128, 260], f32, tag="acc",
                            name=f"att{h}_ps", bufs=4)
                    for h in range(2)
                ]
                for kj in range(njk):
                    if kj in fillers:
                        fillers[kj]()
                    q_lo = max(gq * 512, kj * 128)
                    W = gq * 512 + 512 - q_lo
                    qo = q_lo - gq * 512
                    sc_ps = ps.tile([128, 1024], f32, tag="scores",
                                    name="sc_ps")
                    ex_sb = sb3.tile([128, 1024], f16, tag="expT",
                                     name="ex_sb")
                    for h in range(2):
                        # scoresT[k, q] = kT-chunk contracted with qT
                        nc.tensor.matmul(
                            sc_ps[:, h * 512 + qo : h * 512 + qo + W],
                            qkvT2_sb[h * 64 : (h + 1) * 64,
                                     S + kj * 128 : S + (kj + 1) * 128],
                            qkvT2_sb[h * 64 : (h + 1) * 64, q_lo : q_lo + W],
                            start=True,
                            stop=True,
                            tile_position=(h * 64, 0),
                        )
                    # exp over both heads in one ACT instruction; the first
                    # chunk of each block is split in half so its first PV
                    # sub-chunks start ~0.5 us sooner (pipeline fill)
                    exv = ex_sb.rearrange("p (h q) -> p h q", h=2)
                    scv = sc_ps.rearrange("p (h q) -> p h q", h=2)
                    if kj == 0:
                        nc.scalar.activation(
                            exv[:, :, 0:256], scv[:, :, 0:256], Exp,
                            scale=SCALE)
                        nc.scalar.activation(
                            exv[:, :, 256:512], scv[:, :, 256:512], Exp,
                            scale=SCALE)
                    else:
                        nc.scalar.activation(
                            exv[:, :, qo : qo + W], scv[:, :, qo : qo + W],
                            Exp, scale=SCALE)
                    if kj >= 4 * gq:  # diagonal chunk: zero out k > q
                        exv = ex_sb.rearrange("p (h q) -> p h q", h=2)
                        nc.vector.tensor_mul(
                            exv[:, :, qo : qo + 128],
                            exv[:, :, qo : qo + 128],
                            trimask_sb.rearrange("p (h q) -> p h q", h=2),
                        )
                    # att_q[q, d_aug] += ex[k, q]^T-contracted with v_aug
                    # (ex chunk is the stationary; LDWEIGHTS overlaps).
                    # One accumulation group per tile: PSUM groups are
                    # per-bank, so the sub-chunk column slices share a
                    # single start/stop window.
                    for h in range(2):
                        for c in range(max(0, kj - 4 * gq), 4):
                            nc.tensor.matmul(
                                att_ps[h][:, c * 65 : c * 65 + 65],
                                ex_sb[:, h * 512 + c * 128
                                      : h * 512 + (c + 1) * 128],
                                v2_sb[:, kj * 160 + h * 80
                                      : kj * 160 + h * 80 + 65],
                                start=(kj == 0 and c == 0),
                                stop=(kj == njk - 1 and c == 3),
                            )
                # normalize: stage PSUM->SBUF in one copy (frees the 'acc'
                # ring, which gates the next qkv groups, ~1 us sooner), then
                # per-partition reciprocal + scalar muls into attQ
                # [q, c*128 + h*64 + d] fp16
                for h in range(2):
                    st = sb.tile([128, 260], f32, tag=f"st{h}",
                                 name=f"st{h}")
                    nc.vector.tensor_copy(st, att_ps[h])
                    rr4 = sb.tile([128, 4], f32, tag=f"rr{h}", name=f"rr{h}")
                    nc.vector.reciprocal(
                        rr4, st.rearrange("p (c t) -> p c t", t=65)[:, :, 64]
                    )
                    for c in range(4):
                        nc.vector.tensor_scalar_mul(
                            attQ_sb[:, gq * 512 + c * 128 + h * 64
                                    : gq * 512 + c * 128 + h * 64 + 64],
                            st[:, c * 65 : c * 65 + 64],
                            rr4[:, c : c + 1],
                        )
                # (the attQ->attT transpose and attB copies are emitted
                # later via emit_attn_finish, so they never park ahead of
                # the next v2 transpose on the SP ring)

            def emit_attn_finish(attQ_sb, attT_sb, attB, gq, after=()):
                # transpose attQ block -> attT [h*64+d, q] via the DMA xbar.
                # `after`: nosync deps forcing the scheduler to place this
                # chain behind the given v2 transposes on the SP ring (it
                # would otherwise hoist it ahead of them, and the transpose
                # guard would stall PV on v2 for ~8 us).
                attT3 = attT_sb.rearrange("p (c t) -> p c t", t=128)
                ti = nc.sync.dma_start(
                    attT3[:, 4 * gq : 4 * gq + 4, :],
                    attQ_sb[:, gq * 512 : (gq + 1) * 512],
                    transpose=True,
                )
                if after:
                    deps = InstructionNameOrderedSet()
                    for bi in after:
                        deps.add(bi.ins.name)
                    ti.ins.add_nosync_dependencies_from(deps)
                # o_proj operand (attB cols h*S+q): top = attn_h[d, q],
                # bottom = attn_h[d, q+1] (shifted), so K=128 w-pair chunks
                # read with one stride-16 AP.  Block gq enables shifted-dest
                # columns [gq*512-1, gq*512+511).
                lo = gq * 512
                src_lo = max(1, lo)
                for h in range(2):
                    nc.sync.dma_start(
                        attB[0:64, h * S + lo : h * S + lo + 512],
                        attT_sb[h * 64 : (h + 1) * 64, lo : lo + 512],
                    )
                    nc.sync.dma_start(
                        attB[64:128, h * S + src_lo - 1 : h * S + lo + 511],
                        attT_sb[h * 64 : (h + 1) * 64, src_lo : lo + 512],
                    )

            def emit_oproj_group(b, attB, out_sbs, h, n2):
                # out_band[u, c] = sum_j sum_{k<128}
                #   attB[k, h*S + u*16+2j] Wo[128j+k, c]   (K=128 per matmul)
                attv = attB[:, h * S : (h + 1) * S].rearrange(
                    "p (u w) -> p w u", w=16)
                po = ps.tile([128, 512], f32, tag="acc", name="po", bufs=4)
                for j in range(8):
                    nc.tensor.matmul(
                        po,
                        attv[:, 2 * j, :],
                        wo8_sb[:, j * E + n2 * 512 : j * E + n2 * 512 + 512],
                        start=(j == 0),
                        stop=False,
                    )
                # bias row via K=1 ones matmul
                nc.tensor.matmul(
                    po,
                    ones_sb[0:1, :],
                    bo_sb[0:1, n2 * 512 : (n2 + 1) * 512],
                    start=False,
                    stop=True,
                )
                nc.vector.tensor_copy(
                    out_sbs[h][:, n2 * 512 : (n2 + 1) * 512], po
                )
                if n2 == 1:
                    nc.sync.dma_start(out.ap()[b, h], out_sbs[h])

            # schedule per batch: [qkv n0, qkv n1, gq0, qkv n2, gq1, qkv n3,
            # gq2, gq3] with o_proj(b-1) groups woven into the ACT-bound
            # gq2/gq3 regions so PE never drains while ACT catches up.
            prev = None
            fin_args = None
            for b in range(B):
                xt_sb = emit_xload(b, 0, 1)
                qkvT2_sb = sb.tile([128, 3 * S], f16, tag="qkvT2")
                # v2 chunk layout (stride 160):
                #   [v_h0(64) | ones | pad15 | v_h1(64) | ones | pad15]
                v2_sb = sb.tile([128, 160 * (S // 128)], f16, tag="v2")
                v2v = v2_sb.rearrange("p (c t) -> p c t", t=160)
                nc.gpsimd.memset(v2v[:, :, 64:65], 1.0)
                nc.gpsimd.memset(v2v[:, :, 144:145], 1.0)
                attQ_sb = sb.tile([128, S], f16, tag="attQ", name="attQ")
                attT_sb = sb.tile([128, S], f16, tag="attT", name="attT")
                attB = sb.tile([128, 2 * S], f16, tag="attB", name="attB")

                vts = {}

                def qkv_m(n, m, vt=False):
                    def fn():
                        emit_qkv_mgroup(xt_sb, qkvT2_sb, n, m)
                        if vt:
                            vts[n] = emit_vtrans(qkvT2_sb, v2v, n)
                    return fn

                def attn(gq, fillers=()):
                    emit_attn_gq(qkvT2_sb, v2_sb, attQ_sb, attT_sb, attB,
                                 gq, fillers)

                def finish(gq, after_n):
                    def fn():
                        emit_attn_finish(attQ_sb, attT_sb, attB, gq,
                                         after=vts.get(after_n, ()))
                    return fn

                # v-group first so the v2 transpose (2.2 us DMA latency)
                # overlaps the q/k groups instead of stalling gq0's PV
                emit_qkv_mgroup(xt_sb, qkvT2_sb, 0, 2)
                vts[0] = emit_vtrans(qkvT2_sb, v2v, 0)
                if b == 0:
                    emit_xload(0, 1, 4, xt_sb)  # rest of batch-0 x after vt0
                if fin_args is not None:
                    # deferred gq2-finish of b-1, ordered behind vt0
                    emit_attn_finish(*fin_args, 2, after=vts[0])
                emit_qkv_mgroup(xt_sb, qkvT2_sb, 0, 0)
                emit_qkv_mgroup(xt_sb, qkvT2_sb, 0, 1)
                emit_qkv_mgroup(xt_sb, qkvT2_sb, 1, 0)
                emit_qkv_mgroup(xt_sb, qkvT2_sb, 1, 1)
                # filler layout follows the 'acc' PSUM ring (4 bufs): at
                # most two pq/po groups inside each attention block (their
                # ring gates resolve pre-block), one right after it, rest in
                # the head.  finish() DMAs allocate no PSUM; each is nosync-
                # ordered behind the next v2-transpose pair so the scheduler
                # never parks its copy chain ahead of them on the SP ring.
                f0 = [(1, qkv_m(1, 2, vt=True)), (2, qkv_m(2, 0))]
                if fin_args is not None:
                    fa = fin_args
                    f0.append((3, lambda: emit_attn_finish(
                        *fa, 3, after=vts[1])))
                attn(0, f0)
                emit_qkv_mgroup(xt_sb, qkvT2_sb, 2, 1)  # post-gq0
                if b == 0:
                    emit_wo8(0, 8)  # ACT ring; emitted after batch 0's
                    # first attention block so the transfers never sit in
                    # flight ahead of the startup v2 transposes
                attn(1, [(2, qkv_m(2, 2, vt=True)), (5, qkv_m(3, 0)),
                         (6, finish(0, 2))])
                emit_qkv_mgroup(xt_sb, qkvT2_sb, 3, 1)  # post-gq1
                if prev is not None:
                    pb, pattB, pout = prev
                    attn(2, [(2, qkv_m(3, 2, vt=True)), (4, finish(1, 3)),
                             (5, lambda: emit_oproj_group(
                                 pb, pattB, pout, 0, 0))])
                    emit_oproj_group(pb, pattB, pout, 0, 1)  # post-gq2
                    attn(3, [(2, lambda: emit_oproj_group(
                                 pb, pattB, pout, 1, 0)),
                             (5, lambda: emit_oproj_group(
                                 pb, pattB, pout, 1, 1))])
                else:
                    attn(2, [(2, qkv_m(3, 2, vt=True)), (4, finish(1, 3))])
                    attn(3)
                out_sbs = [
                    sb.tile([128, E], f16, tag=f"outsb{h}", name=f"out{h}_sb")
                    for h in range(2)
                ]
                prev = (b, attB, out_sbs)
                fin_args = (attQ_sb, attT_sb, attB)
            pb, pattB, pout = prev
            emit_attn_finish(*fin_args, 2)
            emit_attn_finish(*fin_args, 3)
            for h in range(2):
                for n2 in range(2):
                    emit_oproj_group(pb, pattB, pout, h, n2)

    nc.compile()
    return nc


def _get_program(dbg=False):
    key = ("nc",)
    if key not in _CACHE:
        _CACHE[key] = _build_program()
    return _CACHE[key]


def _host_inputs(x, Wqkv, bqkv, Wo, bo):
    """Build per-core input maps (host-side layout prep: cast/slice/transpose)."""
    xT = np.ascontiguousarray(x.transpose(0, 2, 1)).astype(np.float16)

    wo8 = np.ascontiguousarray(
        Wo.astype(np.float16).reshape(8, 128, E)
    )
    bo1 = bo.astype(np.float16)[None, :]

    k_idx = np.arange(128)[:, None]
    q_idx = np.arange(128)[None, :]
    tri = (k_idx <= q_idx).astype(np.float16)
    trimask = np.concatenate([tri, tri], axis=1)  # one copy per head

    in_maps = []
    for c in range(NCORES):
        cols = []
        for off in (0, 64, 128):  # q, k, v
            for h in (HPC * c, HPC * c + 1):
                cols.extend(range(h * 3 * D + off, h * 3 * D + off + 64))
        cols = np.asarray(cols)
        in_maps.append(
            {
                "xT": xT,
                "wqkv": np.ascontiguousarray(Wqkv[:, cols]).astype(np.float16),
                "bqkv": np.ascontiguousarray(
                    bqkv[cols].reshape(3, 128).T
                ).astype(np.float32),
                "wo8": wo8,
                "bo1": bo1,
                "trimask": trimask,
            }
        )
    return in_maps


def kernel(x, mask, Wqkv, bqkv, Wo, bo, _n_cores=NCORES, _trace=False, _dbg=False):
    """Full-input, full-output MHA. `mask` is the causal tril mask (hardcoded)."""
    from concourse.bass_utils import run_bass_kernel_spmd

    nc = _get_program()
    in_maps = _host_inputs(
        np.asarray(x), np.asarray(Wqkv), np.asarray(bqkv), np.asarray(Wo), np.asarray(bo)
    )[:_n_cores]
    res = run_bass_kernel_spmd(
        nc, in_maps, core_ids=list(range(_n_cores)), trace=_trace
    )
    out_full = np.zeros((B, S, E), np.float32)
    for c in range(_n_cores):
        o = res.results[c]["out"]  # [B, HPC, 128, E]
        for h in range(HPC):
            g = HPC * c + h
            out_full[:, g * 128 : (g + 1) * 128, :] = o[:, h]
    _CACHE["last_results"] = res
    return out_full
